# revision 14
# baseline (speedup 1.0000x reference)
"""Chamfer loss kernel for Trainium2 (8 NeuronCores, data-parallel over batch).

Math:
  For each batch b: P[i,j] = |x_i|^2 + |y_j|^2 - 2 x_i.y_j  (x=preds[b].T, y=gts[b].T)
  loss_b = sum_j min_i P + sum_i min_j P ; output = sum_b loss_b.

  On device we compute Z = x.y - |y|^2/2 via a K=11 matmul:
    lhsT rows: [hx0..hx2, hx0..hx2, lx0..lx2, 1, 1]
    rhs  rows: [hy0..hy2, ly0..ly2, hy0..hy2, -hsqy/2, -lsqy/2]
  (bf16 hi/lo pairs give exact cross products; the x.y error from the
  dropped lo.lo term is ~2^-18 relative.)
  The per-row term -|x|^2/2 is applied as a per-partition bias during the
  PSUM exit (ACT activation bias / Pool tensor_scalar), yielding
    s = x.y - |x|^2/2 - |y|^2/2 = -P/2  in fp16.
  min_i P = -2 max_i s, so loss_b = -2 * (sum_j max_i s + sum_i max_j s).

Engine balance (per i-block: 4 PSUM quads = [128, 8192] of Z):
  ACT : fp32->fp16 biased exits of ~7.5 of every 8 quads (2-block cycle)
  Pool: the remaining half-quad exit + col-merge of j in [4096, 8192)
  DVE : col-merge of j in [0, 4096) (fp16 2x) + row pass = one [128,4096]
        TT-max (2x) + one TensorTensorReduce (merge + row-reduce + accum)
  PE  : 4 wide 2048-col matmuls per block; PSUM quads free right after the
        exits so the PE stays busy and ramps to the 2.4 GHz p-state.
"""

import os
from contextlib import ExitStack

import numpy as np

import concourse.bacc as bacc
import concourse.bass as bass
import concourse.mybir as mybir
import concourse.tile as tile
from concourse.bass_utils import run_bass_kernel_spmd

B, D, N = 8, 3, 8192
N_CORES = 8

IB = 128          # i-block (output partition dim)
QW = 2048         # PSUM quad width (4 banks, fp32)
N_IB = N // IB    # 64
N_Q = N // QW     # 4

F32 = mybir.dt.float32
F16 = mybir.dt.float16
BF16 = mybir.dt.bfloat16
AX = mybir.AxisListType
ALU = mybir.AluOpType
AF = mybir.ActivationFunctionType

_last_results = None  # stash for test harness (exec_time etc.)


def build_kernel(n: int = N):
    """Builds the SPMD Bass program for one core handling one batch."""
    n_ib = n // IB
    n_q = n // QW

    nc = bacc.Bacc("TRN2", target_bir_lowering=False, debug=False)

    preds_d = nc.dram_tensor("preds", [D, n], F32, kind="ExternalInput").ap()
    gts_d = nc.dram_tensor("gts", [D, n], F32, kind="ExternalInput").ap()
    ident_d = nc.dram_tensor("ident", [128, 128], F16, kind="ExternalInput").ap()
    identf_d = nc.dram_tensor("identf", [32, 32], F32, kind="ExternalInput").ap()
    out_d = nc.dram_tensor("out", [1, 1], F32, kind="ExternalOutput").ap()

    with tile.TileContext(nc) as tc, ExitStack() as ctx:
        persist = ctx.enter_context(tc.tile_pool(name="persist", bufs=1))
        spool = ctx.enter_context(tc.tile_pool(name="spool", bufs=3))
        rpool = ctx.enter_context(tc.tile_pool(name="rpool", bufs=2))

        # ---- persistent tensors ----
        XT = persist.tile([11, n], BF16)
        YT = persist.tile([11, n], BF16)
        ident = persist.tile([128, 128], F16)
        rxh = persist.tile([128, n_ib], F32)     # -|x_i|^2/2, i-major [128, 64]
        C = persist.tile([128, n], F16)          # col-max accumulator over i
        rowmaxes = persist.tile([128, n_ib], F32)
        nc.sync.dma_start(ident[:], ident_d[:])
        # col accumulator starts at -inf (overlaps the prologue DMAs)
        nc.vector.memset(C[:], float(np.finfo(np.float16).min))

        # ---- prologue ----
        # Math runs in a [96, n/32] layout (partition p = d*32 + c, chunk c
        # of 32) so all lanes are used; DMAs scatter rows into place after.
        fw = n // 32
        with tc.tile_pool(name="propool", bufs=1) as pro:
            # x side: hi/lo split only (no squares; rx handled via bias)
            Px = pro.tile([96, fw], F32)
            Hx = pro.tile([96, fw], BF16)
            Lx = pro.tile([96, fw], BF16)
            nc.sync.dma_start(Px[:], preds_d.rearrange("d (c f) -> (d c) f", c=32))
            nc.scalar.copy(Hx[:], Px[:])
            nc.vector.tensor_tensor(out=Lx[:], in0=Px[:], in1=Hx[:], op=ALU.subtract)

            # y side: hi/lo split + summed squares scaled by -1/2
            Py = pro.tile([96, fw], F32)
            Hy = pro.tile([96, fw], BF16)
            Ly = pro.tile([96, fw], BF16)
            nc.sync.dma_start(Py[:], gts_d.rearrange("d (c f) -> (d c) f", c=32))
            nc.scalar.copy(Hy[:], Py[:])
            nc.vector.tensor_tensor(out=Ly[:], in0=Py[:], in1=Hy[:], op=ALU.subtract)
            # engines need all operands on the same base partition, so the
            # per-d sum runs in a [32, (d, fw)] layout loaded separately
            Yd = pro.tile([32, 3 * fw], F32)
            for d in range(D):
                nc.sync.dma_start(
                    Yd[:, d * fw:(d + 1) * fw],
                    gts_d[d:d + 1, :].rearrange("o (c f) -> (o c) f", c=32),
                )
            SQ = pro.tile([32, 3 * fw], F32)
            SY = pro.tile([32, fw], F32)
            S2 = pro.tile([32, fw], F32)
            HS = pro.tile([32, fw], BF16)
            LS = pro.tile([32, fw], BF16)
            nc.vector.tensor_tensor(out=SQ[:], in0=Yd[:], in1=Yd[:], op=ALU.mult)
            nc.vector.tensor_reduce(
                out=SY[:], in_=SQ[:].rearrange("p (d f) -> p f d", d=3),
                axis=AX.X, op=ALU.add,
            )
            nc.scalar.mul(S2[:], SY[:], -0.5)
            nc.scalar.copy(HS[:], S2[:])
            nc.vector.tensor_tensor(out=LS[:], in0=S2[:], in1=HS[:], op=ALU.subtract)

            # ones rows for the -|y|^2/2 rank-1 terms
            ONE = pro.tile([64, fw], BF16)
            nc.gpsimd.memset(ONE[:], 1.0)

            # scatter into XT/YT row layout (batched 3-row DMAs)
            def scat(T, r0, nrows, src):
                nc.sync.dma_start(
                    T[r0:r0 + nrows, :].rearrange("p (c f) -> p c f", c=32),
                    src,
                )
            scat(XT, 0, 3, Hx[:])
            scat(XT, 3, 3, Hx[:])
            scat(XT, 6, 3, Lx[:])
            scat(XT, 9, 2, ONE[:])
            scat(YT, 0, 3, Hy[:])
            scat(YT, 3, 3, Ly[:])
            scat(YT, 6, 3, Hy[:])
            scat(YT, 9, 1, HS[:])
            scat(YT, 10, 1, LS[:])

            # rxh[p, b] = -|x_{b*128+p}|^2/2  (i-major for per-partition bias).
            # Built via PE transpose: |x|^2 in the chunk layout [32, 256]
            # (i = c*256 + f), transposed 128-col halves give [128, 32]
            # tiles whose cols are even/odd block indices.
            Xd = pro.tile([32, 3 * fw], F32)
            for d in range(D):
                nc.sync.dma_start(
                    Xd[:, d * fw:(d + 1) * fw],
                    preds_d[d:d + 1, :].rearrange("o (c f) -> (o c) f", c=32),
                )
            XSQ = pro.tile([32, 3 * fw], F32)
            RXS = pro.tile([32, fw], F32)
            nc.vector.tensor_tensor(out=XSQ[:], in0=Xd[:], in1=Xd[:], op=ALU.mult)
            nc.vector.tensor_reduce(
                out=RXS[:], in_=XSQ[:].rearrange("p (d f) -> p f d", d=3),
                axis=AX.X, op=ALU.add,
            )
            identf = pro.tile([32, 32], F32)
            nc.sync.dma_start(identf[:], identf_d[:])
            with tc.tile_pool(name="prot", bufs=1, space=bass.MemorySpace.PSUM) as prot:
                for h in range(2):
                    pt = prot.tile([128, 32], F32, name=f"pt{h}")
                    nc.tensor.transpose(
                        pt[:], RXS[:, h * 128:(h + 1) * 128], identf[:])
                    nc.scalar.mul(
                        rxh[:].rearrange("p (b two) -> p two b", two=2)[:, h, :],
                        pt[:], -0.5)

        # ---- main loop ----
        psum_ctx = tc.tile_pool(name="psum", bufs=2, space=bass.MemorySpace.PSUM)
        psum = psum_ctx.__enter__()
        NEG = float(-3e38)
        for ib in range(n_ib):
            lhsT = XT[:, ib * IB:(ib + 1) * IB]
            bias = rxh[:, ib:ib + 1]
            s = spool.tile([128, n], F16, tag="s")
            for q in range(n_q):
                p = psum.tile([128, QW], F32, tag="p")
                for m in range(QW // 512):
                    c0 = q * QW + m * 512
                    nc.tensor.matmul(
                        p[:, m * 512:(m + 1) * 512], lhsT, YT[:, c0:c0 + 512],
                        start=True, stop=True,
                    )
                # ACT exits the quad (Pool can't read PSUM on TRN2)
                nc.scalar.activation(
                    s[:, q * QW:(q + 1) * QW], p[:],
                    AF.Identity, bias=bias, scale=1.0,
                )
            # col accumulator: one wide fp16 2x merge (DVE is the only
            # engine that can do bulk max on TRN2; Pool TT doesn't lower)
            nc.vector.tensor_tensor(
                out=C[:], in0=C[:], in1=s[:], op=ALU.max)
            # row pass: wide 2x TT tree, fold, then a 1x reduce
            R = rpool.tile([128, 2 * QW], F16, tag="R")
            nc.vector.tensor_tensor(
                out=R[:], in0=s[:, 0:2 * QW], in1=s[:, 2 * QW:4 * QW], op=ALU.max)
            nc.vector.tensor_tensor(
                out=R[:, 0:QW], in0=R[:, 0:QW], in1=R[:, QW:2 * QW], op=ALU.max)
            nc.vector.tensor_reduce(
                out=rowmaxes[:, ib:ib + 1], in_=R[:, 0:QW], axis=AX.X, op=ALU.max)

        psum_ctx.__exit__(None, None, None)

        # ---- tails ----
        tailp = ctx.enter_context(
            tc.tile_pool(name="tailp", bufs=2, space=bass.MemorySpace.PSUM)
        )
        # loss2 partial: sum_i max_j  -> [128,1]
        acc2 = persist.tile([128, 1], F32)
        nc.vector.reduce_sum(out=acc2[:], in_=rowmaxes[:], axis=AX.X)

        # loss1: partition-max of every C column via PE transpose (4 chunks
        # batched per PSUM tile, one [128, 4, 128] reduce each), then sum_j
        n_cols = n // 128
        colmax_cols = persist.tile([128, n_cols], F32)
        for g in range(n_cols // 4):
            pt = tailp.tile([128, 512], F16, tag="pt")
            for c in range(4):
                ch = g * 4 + c
                nc.tensor.transpose(
                    pt[:, c * 128:(c + 1) * 128],
                    C[:, ch * 128:(ch + 1) * 128], ident[:],
                )
            nc.vector.tensor_reduce(
                out=colmax_cols[:, g * 4:g * 4 + 4],
                in_=pt[:].rearrange("p (c f) -> p c f", c=4),
                axis=AX.X, op=ALU.max,
            )
        acc1 = persist.tile([128, 1], F32)
        nc.vector.reduce_sum(out=acc1[:], in_=colmax_cols[:], axis=AX.X)

        total = persist.tile([128, 1], F32)
        nc.vector.tensor_tensor(out=total[:], in0=acc1[:], in1=acc2[:], op=ALU.add)

        # partition-sum via matmul with ones, then scale by -2
        ones = persist.tile([128, 1], F32)
        nc.vector.memset(ones[:], 1.0)
        ps = tailp.tile([1, 1], F32, tag="ps")
        nc.tensor.matmul(ps[:], ones[:], total[:], start=True, stop=True)
        out_sb = persist.tile([1, 1], F32)
        nc.scalar.mul(out_sb[:], ps[:], -2.0)
        nc.sync.dma_start(out_d[:], out_sb[:])

    nc.compile()
    return nc


def kernel(preds: np.ndarray, gts: np.ndarray) -> np.ndarray:
    global _last_results
    assert preds.shape == (B, D, N) and gts.shape == (B, D, N)
    nc = build_kernel(N)
    eye = np.eye(128, dtype=np.float16)
    in_maps = [
        {
            "preds": np.ascontiguousarray(preds[b], dtype=np.float32),
            "gts": np.ascontiguousarray(gts[b], dtype=np.float32),
            "ident": eye,
            "identf": np.eye(32, dtype=np.float32),
        }
        for b in range(N_CORES)
    ]
    res = run_bass_kernel_spmd(
        nc,
        in_maps,
        core_ids=list(range(N_CORES)),
        trace=bool(os.environ.get("BASS_TRACE")),
    )
    _last_results = res
    total = sum(float(res.results[i]["out"].reshape(-1)[0]) for i in range(N_CORES))
    return np.array(total, dtype=np.float32)


# revision 18
# speedup vs baseline: 1.0501x; 1.0501x over previous
"""Chamfer loss kernel for Trainium2 (8 NeuronCores, data-parallel over batch).

Math:
  For each batch b: P[i,j] = |x_i|^2 + |y_j|^2 - 2 x_i.y_j  (x=preds[b].T, y=gts[b].T)
  loss_b = sum_j min_i P + sum_i min_j P ; output = sum_b loss_b.

  On device we compute Z = x.y - |y|^2/2 via a K=11 matmul:
    lhsT rows: [hx0..hx2, hx0..hx2, lx0..lx2, 1, 1]
    rhs  rows: [hy0..hy2, ly0..ly2, hy0..hy2, -hsqy/2, -lsqy/2]
  (bf16 hi/lo pairs give exact cross products; the x.y error from the
  dropped lo.lo term is ~2^-18 relative.)
  The per-row term -|x|^2/2 is applied as a per-partition bias during the
  PSUM exit (ACT activation bias / Pool tensor_scalar), yielding
    s = x.y - |x|^2/2 - |y|^2/2 = -P/2  in fp16.
  min_i P = -2 max_i s, so loss_b = -2 * (sum_j max_i s + sum_i max_j s).

Engine balance (per i-block: 4 PSUM quads = [128, 8192] of Z):
  ACT : fp32->fp16 biased exits of ~7.5 of every 8 quads (2-block cycle)
  Pool: the remaining half-quad exit + col-merge of j in [4096, 8192)
  DVE : col-merge of j in [0, 4096) (fp16 2x) + row pass = one [128,4096]
        TT-max (2x) + one TensorTensorReduce (merge + row-reduce + accum)
  PE  : 4 wide 2048-col matmuls per block; PSUM quads free right after the
        exits so the PE stays busy and ramps to the 2.4 GHz p-state.
"""

import os
from contextlib import ExitStack

import numpy as np

import concourse.bacc as bacc
import concourse.bass as bass
import concourse.mybir as mybir
import concourse.tile as tile
from concourse.bass_utils import run_bass_kernel_spmd

B, D, N = 8, 3, 8192
N_CORES = 8

IB = 128          # i-block (output partition dim)
QW = 2048         # PSUM quad width (4 banks, fp32)
N_IB = N // IB    # 64
N_Q = N // QW     # 4

F32 = mybir.dt.float32
F16 = mybir.dt.float16
BF16 = mybir.dt.bfloat16
AX = mybir.AxisListType
ALU = mybir.AluOpType
AF = mybir.ActivationFunctionType

_last_results = None  # stash for test harness (exec_time etc.)


def build_kernel(n: int = N):
    """Builds the SPMD Bass program for one core handling one batch."""
    n_ib = n // IB
    n_q = n // QW

    nc = bacc.Bacc("TRN2", target_bir_lowering=False, debug=False)

    preds_d = nc.dram_tensor("preds", [D, n], F32, kind="ExternalInput").ap()
    gts_d = nc.dram_tensor("gts", [D, n], F32, kind="ExternalInput").ap()
    ident_d = nc.dram_tensor("ident", [128, 128], F16, kind="ExternalInput").ap()
    identf_d = nc.dram_tensor("identf", [32, 32], F32, kind="ExternalInput").ap()
    out_d = nc.dram_tensor("out", [1, 1], F32, kind="ExternalOutput").ap()

    with tile.TileContext(nc) as tc, ExitStack() as ctx:
        persist = ctx.enter_context(tc.tile_pool(name="persist", bufs=1))
        spool = ctx.enter_context(tc.tile_pool(name="spool", bufs=3))
        rpool = ctx.enter_context(tc.tile_pool(name="rpool", bufs=2))

        # ---- persistent tensors ----
        XT = persist.tile([11, n], BF16)
        YT = persist.tile([11, n], BF16)
        ident = persist.tile([128, 128], F16)
        rxh = persist.tile([128, n_ib], F32)     # -|x_i|^2/2, i-major [128, 64]
        C = persist.tile([128, n], F16)          # col-max accumulator over i
        rowmaxes = persist.tile([128, n_ib], F32)
        nc.sync.dma_start(ident[:], ident_d[:])
        # col accumulator starts at -inf (overlaps the prologue DMAs)
        nc.vector.memset(C[:], float(np.finfo(np.float16).max))

        # ---- prologue ----
        # Math runs in a [96, n/32] layout (partition p = d*32 + c, chunk c
        # of 32) so all lanes are used; DMAs scatter rows into place after.
        fw = n // 32
        with tc.tile_pool(name="propool", bufs=1) as pro:
            # x side: hi/lo split only (no squares; rx handled via bias)
            Px = pro.tile([96, fw], F32)
            Hx = pro.tile([96, fw], BF16)
            Lx = pro.tile([96, fw], BF16)
            nc.sync.dma_start(Px[:], preds_d.rearrange("d (c f) -> (d c) f", c=32))
            nc.scalar.copy(Hx[:], Px[:])
            nc.vector.tensor_tensor(out=Lx[:], in0=Px[:], in1=Hx[:], op=ALU.subtract)

            # y side: hi/lo split + summed squares scaled by -1/2
            Py = pro.tile([96, fw], F32)
            Hy = pro.tile([96, fw], BF16)
            Ly = pro.tile([96, fw], BF16)
            nc.sync.dma_start(Py[:], gts_d.rearrange("d (c f) -> (d c) f", c=32))
            nc.scalar.copy(Hy[:], Py[:])
            nc.vector.tensor_tensor(out=Ly[:], in0=Py[:], in1=Hy[:], op=ALU.subtract)
            # engines need all operands on the same base partition, so the
            # per-d sum runs in a [32, (d, fw)] layout loaded separately
            Yd = pro.tile([32, 3 * fw], F32)
            for d in range(D):
                nc.sync.dma_start(
                    Yd[:, d * fw:(d + 1) * fw],
                    gts_d[d:d + 1, :].rearrange("o (c f) -> (o c) f", c=32),
                )
            SQ = pro.tile([32, 3 * fw], F32)
            SY = pro.tile([32, fw], F32)
            S2 = pro.tile([32, fw], F32)
            HS = pro.tile([32, fw], BF16)
            LS = pro.tile([32, fw], BF16)
            nc.vector.tensor_tensor(out=SQ[:], in0=Yd[:], in1=Yd[:], op=ALU.mult)
            nc.vector.tensor_reduce(
                out=SY[:], in_=SQ[:].rearrange("p (d f) -> p f d", d=3),
                axis=AX.X, op=ALU.add,
            )
            nc.scalar.mul(S2[:], SY[:], -0.5)
            nc.scalar.copy(HS[:], S2[:])
            nc.vector.tensor_tensor(out=LS[:], in0=S2[:], in1=HS[:], op=ALU.subtract)

            # ones rows for the -|y|^2/2 rank-1 terms
            ONE = pro.tile([64, fw], BF16)
            nc.gpsimd.memset(ONE[:], 1.0)

            # scatter into XT/YT row layout (batched 3-row DMAs)
            def scat(T, r0, nrows, src):
                nc.sync.dma_start(
                    T[r0:r0 + nrows, :].rearrange("p (c f) -> p c f", c=32),
                    src,
                )
            scat(XT, 0, 3, Hx[:])
            scat(XT, 3, 3, Hx[:])
            scat(XT, 6, 3, Lx[:])
            scat(XT, 9, 2, ONE[:])
            scat(YT, 0, 3, Hy[:])
            scat(YT, 3, 3, Ly[:])
            scat(YT, 6, 3, Hy[:])
            scat(YT, 9, 1, HS[:])
            scat(YT, 10, 1, LS[:])

            # rxh[p, b] = -|x_{b*128+p}|^2/2  (i-major for per-partition bias).
            # Built via PE transpose: |x|^2 in the chunk layout [32, 256]
            # (i = c*256 + f), transposed 128-col halves give [128, 32]
            # tiles whose cols are even/odd block indices.
            Xd = pro.tile([32, 3 * fw], F32)
            for d in range(D):
                nc.sync.dma_start(
                    Xd[:, d * fw:(d + 1) * fw],
                    preds_d[d:d + 1, :].rearrange("o (c f) -> (o c) f", c=32),
                )
            XSQ = pro.tile([32, 3 * fw], F32)
            RXS = pro.tile([32, fw], F32)
            nc.vector.tensor_tensor(out=XSQ[:], in0=Xd[:], in1=Xd[:], op=ALU.mult)
            nc.vector.tensor_reduce(
                out=RXS[:], in_=XSQ[:].rearrange("p (d f) -> p f d", d=3),
                axis=AX.X, op=ALU.add,
            )
            identf = pro.tile([32, 32], F32)
            nc.sync.dma_start(identf[:], identf_d[:])
            with tc.tile_pool(name="prot", bufs=1, space=bass.MemorySpace.PSUM) as prot:
                for h in range(2):
                    pt = prot.tile([128, 32], F32, name=f"pt{h}")
                    nc.tensor.transpose(
                        pt[:], RXS[:, h * 128:(h + 1) * 128], identf[:])
                    nc.scalar.mul(
                        rxh[:].rearrange("p (b two) -> p two b", two=2)[:, h, :],
                        pt[:], 0.5)

        # ---- main loop ----
        psum_ctx = tc.tile_pool(name="psum", bufs=2, space=bass.MemorySpace.PSUM)
        psum = psum_ctx.__enter__()
        NEG = float(60000.0)
        for ib in range(n_ib):
            lhsT = XT[:, ib * IB:(ib + 1) * IB]
            bias = rxh[:, ib:ib + 1]
            s = spool.tile([128, n], F16, tag="s")
            for q in range(n_q):
                p = psum.tile([128, QW], F32, tag="p")
                for m in range(QW // 512):
                    c0 = q * QW + m * 512
                    nc.tensor.matmul(
                        p[:, m * 512:(m + 1) * 512], lhsT, YT[:, c0:c0 + 512],
                        start=True, stop=True,
                    )
                # ACT exits the quad (Pool can't read PSUM on TRN2)
                nc.scalar.activation(
                    s[:, q * QW:(q + 1) * QW], p[:],
                    AF.Identity, bias=bias, scale=-1.0,
                )
            # col accumulator: one wide fp16 2x merge (DVE is the only
            # engine that can do bulk max on TRN2; Pool TT doesn't lower)
            nc.vector.tensor_tensor(
                out=C[:], in0=C[:], in1=s[:], op=ALU.min)
            # row pass: wide 2x TT tree, folds, then a narrow 1x reduce
            R = rpool.tile([128, 2 * QW], F16, tag="R")
            nc.vector.tensor_tensor(
                out=R[:], in0=s[:, 0:2 * QW], in1=s[:, 2 * QW:4 * QW], op=ALU.min)
            nc.vector.tensor_tensor(
                out=R[:, 0:QW], in0=R[:, 0:QW], in1=R[:, QW:2 * QW], op=ALU.min)
            nc.vector.tensor_tensor(
                out=R[:, 0:QW // 2], in0=R[:, 0:QW // 2], in1=R[:, QW // 2:QW],
                op=ALU.min)
            nc.vector.tensor_reduce(
                out=rowmaxes[:, ib:ib + 1], in_=R[:, 0:QW // 2], axis=AX.X,
                op=ALU.min)

        psum_ctx.__exit__(None, None, None)

        # ---- tails ----
        tailp = ctx.enter_context(
            tc.tile_pool(name="tailp", bufs=2, space=bass.MemorySpace.PSUM)
        )
        # loss2 partial: sum_i max_j  -> [128,1]
        acc2 = persist.tile([128, 1], F32)
        nc.vector.reduce_sum(out=acc2[:], in_=rowmaxes[:], axis=AX.X)

        # loss1: partition-max of every C column via PE transpose (4 chunks
        # batched per PSUM tile, one [128, 4, 128] reduce each), then sum_j
        n_cols = n // 128
        colmax_cols = persist.tile([128, n_cols], F32)
        for g in range(n_cols // 4):
            pt = tailp.tile([128, 512], F16, tag="pt")
            for c in range(4):
                ch = g * 4 + c
                nc.tensor.transpose(
                    pt[:, c * 128:(c + 1) * 128],
                    C[:, ch * 128:(ch + 1) * 128], ident[:],
                )
            nc.vector.tensor_reduce(
                out=colmax_cols[:, g * 4:g * 4 + 4],
                in_=pt[:].rearrange("p (c f) -> p c f", c=4),
                axis=AX.X, op=ALU.min,
            )
        acc1 = persist.tile([128, 1], F32)
        nc.vector.reduce_sum(out=acc1[:], in_=colmax_cols[:], axis=AX.X)

        total = persist.tile([128, 1], F32)
        nc.vector.tensor_tensor(out=total[:], in0=acc1[:], in1=acc2[:], op=ALU.add)

        # partition-sum via matmul with ones, then scale by -2
        ones = persist.tile([128, 1], F32)
        nc.vector.memset(ones[:], 1.0)
        ps = tailp.tile([1, 1], F32, tag="ps")
        nc.tensor.matmul(ps[:], ones[:], total[:], start=True, stop=True)
        out_sb = persist.tile([1, 1], F32)
        nc.scalar.mul(out_sb[:], ps[:], 2.0)
        nc.sync.dma_start(out_d[:], out_sb[:])

    nc.compile()
    return nc


def kernel(preds: np.ndarray, gts: np.ndarray) -> np.ndarray:
    global _last_results
    assert preds.shape == (B, D, N) and gts.shape == (B, D, N)
    nc = build_kernel(N)
    eye = np.eye(128, dtype=np.float16)
    in_maps = [
        {
            "preds": np.ascontiguousarray(preds[b], dtype=np.float32),
            "gts": np.ascontiguousarray(gts[b], dtype=np.float32),
            "ident": eye,
            "identf": np.eye(32, dtype=np.float32),
        }
        for b in range(N_CORES)
    ]
    res = run_bass_kernel_spmd(
        nc,
        in_maps,
        core_ids=list(range(N_CORES)),
        trace=bool(os.environ.get("BASS_TRACE")),
    )
    _last_results = res
    total = sum(float(res.results[i]["out"].reshape(-1)[0]) for i in range(N_CORES))
    return np.array(total, dtype=np.float32)


# revision 20
# speedup vs baseline: 1.1285x; 1.0747x over previous
"""Chamfer loss kernel for Trainium2 (8 NeuronCores, data-parallel over batch).

Math:
  For each batch b: P[i,j] = |x_i|^2 + |y_j|^2 - 2 x_i.y_j  (x=preds[b].T, y=gts[b].T)
  loss_b = sum_j min_i P + sum_i min_j P ; output = sum_b loss_b.

  On device we compute Z = x.y - |y|^2/2 via a K=11 matmul:
    lhsT rows: [hx0..hx2, hx0..hx2, lx0..lx2, 1, 1]
    rhs  rows: [hy0..hy2, ly0..ly2, hy0..hy2, -hsqy/2, -lsqy/2]
  (bf16 hi/lo pairs give exact cross products; the x.y error from the
  dropped lo.lo term is ~2^-18 relative.)
  The per-row term -|x|^2/2 is applied as a per-partition bias during the
  PSUM exit (ACT activation bias / Pool tensor_scalar), yielding
    s = x.y - |x|^2/2 - |y|^2/2 = -P/2  in fp16.
  min_i P = -2 max_i s, so loss_b = -2 * (sum_j max_i s + sum_i max_j s).

Engine balance (per i-block: 4 PSUM quads = [128, 8192] of Z):
  ACT : fp32->fp16 biased exits of ~7.5 of every 8 quads (2-block cycle)
  Pool: the remaining half-quad exit + col-merge of j in [4096, 8192)
  DVE : col-merge of j in [0, 4096) (fp16 2x) + row pass = one [128,4096]
        TT-max (2x) + one TensorTensorReduce (merge + row-reduce + accum)
  PE  : 4 wide 2048-col matmuls per block; PSUM quads free right after the
        exits so the PE stays busy and ramps to the 2.4 GHz p-state.
"""

import os
from contextlib import ExitStack

import numpy as np

import concourse.bacc as bacc
import concourse.bass as bass
import concourse.dve_ops as dve_ops
import concourse.mybir as mybir
import concourse.tile as tile
from concourse.bass_utils import dve_ver_for, run_bass_kernel_spmd
from concourse.dve_spec import AluOp, C0, Spec, Src0, Src1, _has_src1, lower, minn
from concourse.dve_uop import DveOpSpec


def _register_tt_min_red():
    """Custom DVE op: out = min(in0, in1); accum_out = min(s0, min(out)).

    One instruction covers the whole row pass (pairwise min of two
    half-row streams fused with the free-axis min-reduce). Registered
    into dve_ops at import; the uop table ships inside the NEFF."""
    name = "TT_MIN_RED_ANT"
    if name in dve_ops._SUB_OPCODE_FOR_NAME:
        return next(op for op in dve_ops.OPS if op.name == name)

    def _ref(in0, in1, c0, c1, c2):
        b = np.minimum(in0.astype(np.float32), in1).astype(np.float32)
        acc = np.minimum(c0, b.reshape(b.shape[0], -1).min(axis=-1, keepdims=True))
        return b, acc

    spec = Spec(body=minn(Src0, Src1), accum=AluOp.MIN, accum_init=C0,
                reference=_ref)
    row = max(dve_ops._SUB_OPCODE_FOR_NAME.values()) + 1
    assert row < 0x20
    shas = {}
    for ver in ("v3", "v4"):
        s = DveOpSpec(name=name, opcode=row, uops=lower(spec, ver=ver),
                      rd1_en=_has_src1(spec))
        shas[ver] = s.sha(ver)
    op = dve_ops.DveOp(name, spec, subdim=False, uops_sha=shas)
    dve_ops.OPS.append(op)
    dve_ops.CUSTOM_DVE_SPECS[name] = spec
    dve_ops._SUB_OPCODE_FOR_NAME[name] = row
    return op


TT_MIN_RED = _register_tt_min_red()

B, D, N = 8, 3, 8192
N_CORES = 8

IB = 128          # i-block (output partition dim)
QW = 2048         # PSUM quad width (4 banks, fp32)
N_IB = N // IB    # 64
N_Q = N // QW     # 4

F32 = mybir.dt.float32
F16 = mybir.dt.float16
BF16 = mybir.dt.bfloat16
AX = mybir.AxisListType
ALU = mybir.AluOpType
AF = mybir.ActivationFunctionType

_last_results = None  # stash for test harness (exec_time etc.)


def build_kernel(n: int = N):
    """Builds the SPMD Bass program for one core handling one batch."""
    n_ib = n // IB
    n_q = n // QW

    nc = bacc.Bacc("TRN2", target_bir_lowering=False, debug=False)

    preds_d = nc.dram_tensor("preds", [D, n], F32, kind="ExternalInput").ap()
    gts_d = nc.dram_tensor("gts", [D, n], F32, kind="ExternalInput").ap()
    ident_d = nc.dram_tensor("ident", [128, 128], F16, kind="ExternalInput").ap()
    identf_d = nc.dram_tensor("identf", [32, 32], F32, kind="ExternalInput").ap()
    out_d = nc.dram_tensor("out", [1, 1], F32, kind="ExternalOutput").ap()

    with tile.TileContext(nc) as tc, ExitStack() as ctx:
        persist = ctx.enter_context(tc.tile_pool(name="persist", bufs=1))
        spool = ctx.enter_context(tc.tile_pool(name="spool", bufs=3))
        rpool = ctx.enter_context(tc.tile_pool(name="rpool", bufs=2))

        # ---- persistent tensors ----
        XT = persist.tile([11, n], BF16)
        YT = persist.tile([11, n], BF16)
        ident = persist.tile([128, 128], F16)
        rxh = persist.tile([128, n_ib], F32)     # -|x_i|^2/2, i-major [128, 64]
        C = persist.tile([128, n], F16)          # col-max accumulator over i
        rowmaxes = persist.tile([128, n_ib], F32)
        nc.sync.dma_start(ident[:], ident_d[:])
        # col accumulator starts at -inf (overlaps the prologue DMAs)
        nc.vector.memset(C[:], float(np.finfo(np.float16).max))

        # ---- prologue ----
        # Math runs in a [96, n/32] layout (partition p = d*32 + c, chunk c
        # of 32) so all lanes are used; DMAs scatter rows into place after.
        fw = n // 32
        with tc.tile_pool(name="propool", bufs=1) as pro:
            # x side: hi/lo split only (no squares; rx handled via bias)
            Px = pro.tile([96, fw], F32)
            Hx = pro.tile([96, fw], BF16)
            Lx = pro.tile([96, fw], BF16)
            nc.sync.dma_start(Px[:], preds_d.rearrange("d (c f) -> (d c) f", c=32))
            nc.scalar.copy(Hx[:], Px[:])
            nc.vector.tensor_tensor(out=Lx[:], in0=Px[:], in1=Hx[:], op=ALU.subtract)

            # y side: hi/lo split + summed squares scaled by -1/2
            Py = pro.tile([96, fw], F32)
            Hy = pro.tile([96, fw], BF16)
            Ly = pro.tile([96, fw], BF16)
            nc.sync.dma_start(Py[:], gts_d.rearrange("d (c f) -> (d c) f", c=32))
            nc.scalar.copy(Hy[:], Py[:])
            nc.vector.tensor_tensor(out=Ly[:], in0=Py[:], in1=Hy[:], op=ALU.subtract)
            # engines need all operands on the same base partition, so the
            # per-d sum runs in a [32, (d, fw)] layout loaded separately
            Yd = pro.tile([32, 3 * fw], F32)
            for d in range(D):
                nc.sync.dma_start(
                    Yd[:, d * fw:(d + 1) * fw],
                    gts_d[d:d + 1, :].rearrange("o (c f) -> (o c) f", c=32),
                )
            SQ = pro.tile([32, 3 * fw], F32)
            SY = pro.tile([32, fw], F32)
            S2 = pro.tile([32, fw], F32)
            HS = pro.tile([32, fw], BF16)
            LS = pro.tile([32, fw], BF16)
            nc.vector.tensor_tensor(out=SQ[:], in0=Yd[:], in1=Yd[:], op=ALU.mult)
            nc.vector.tensor_reduce(
                out=SY[:], in_=SQ[:].rearrange("p (d f) -> p f d", d=3),
                axis=AX.X, op=ALU.add,
            )
            nc.scalar.mul(S2[:], SY[:], -0.5)
            nc.scalar.copy(HS[:], S2[:])
            nc.vector.tensor_tensor(out=LS[:], in0=S2[:], in1=HS[:], op=ALU.subtract)

            # ones rows for the -|y|^2/2 rank-1 terms
            ONE = pro.tile([64, fw], BF16)
            nc.gpsimd.memset(ONE[:], 1.0)

            # scatter into XT/YT row layout (batched 3-row DMAs)
            def scat(T, r0, nrows, src):
                nc.sync.dma_start(
                    T[r0:r0 + nrows, :].rearrange("p (c f) -> p c f", c=32),
                    src,
                )
            scat(XT, 0, 3, Hx[:])
            scat(XT, 3, 3, Hx[:])
            scat(XT, 6, 3, Lx[:])
            scat(XT, 9, 2, ONE[:])
            scat(YT, 0, 3, Hy[:])
            scat(YT, 3, 3, Ly[:])
            scat(YT, 6, 3, Hy[:])
            scat(YT, 9, 1, HS[:])
            scat(YT, 10, 1, LS[:])

            # rxh[p, b] = -|x_{b*128+p}|^2/2  (i-major for per-partition bias).
            # Built via PE transpose: |x|^2 in the chunk layout [32, 256]
            # (i = c*256 + f), transposed 128-col halves give [128, 32]
            # tiles whose cols are even/odd block indices.
            Xd = pro.tile([32, 3 * fw], F32)
            for d in range(D):
                nc.sync.dma_start(
                    Xd[:, d * fw:(d + 1) * fw],
                    preds_d[d:d + 1, :].rearrange("o (c f) -> (o c) f", c=32),
                )
            XSQ = pro.tile([32, 3 * fw], F32)
            RXS = pro.tile([32, fw], F32)
            nc.vector.tensor_tensor(out=XSQ[:], in0=Xd[:], in1=Xd[:], op=ALU.mult)
            nc.vector.tensor_reduce(
                out=RXS[:], in_=XSQ[:].rearrange("p (d f) -> p f d", d=3),
                axis=AX.X, op=ALU.add,
            )
            identf = pro.tile([32, 32], F32)
            nc.sync.dma_start(identf[:], identf_d[:])
            with tc.tile_pool(name="prot", bufs=1, space=bass.MemorySpace.PSUM) as prot:
                for h in range(2):
                    pt = prot.tile([128, 32], F32, name=f"pt{h}")
                    nc.tensor.transpose(
                        pt[:], RXS[:, h * 128:(h + 1) * 128], identf[:])
                    nc.scalar.mul(
                        rxh[:].rearrange("p (b two) -> p two b", two=2)[:, h, :],
                        pt[:], 0.5)

        # ---- main loop ----
        psum_ctx = tc.tile_pool(name="psum", bufs=2, space=bass.MemorySpace.PSUM)
        psum = psum_ctx.__enter__()
        NEG = float(60000.0)
        for ib in range(n_ib):
            lhsT = XT[:, ib * IB:(ib + 1) * IB]
            bias = rxh[:, ib:ib + 1]
            s = spool.tile([128, n], F16, tag="s")
            for q in range(n_q):
                p = psum.tile([128, QW], F32, tag="p")
                for m in range(QW // 512):
                    c0 = q * QW + m * 512
                    nc.tensor.matmul(
                        p[:, m * 512:(m + 1) * 512], lhsT, YT[:, c0:c0 + 512],
                        start=True, stop=True,
                    )
                # ACT exits the quad (Pool can't read PSUM on TRN2)
                nc.scalar.activation(
                    s[:, q * QW:(q + 1) * QW], p[:],
                    AF.Identity, bias=bias, scale=-1.0,
                )
            # col accumulator: one wide fp16 2x merge (DVE is the only
            # engine that can do bulk max on TRN2; Pool TT doesn't lower)
            nc.vector.tensor_tensor(
                out=C[:], in0=C[:], in1=s[:], op=ALU.min)
            # row pass: one custom DVE op (pairwise min + min-reduce accum)
            R = rpool.tile([128, 2 * QW], F16, tag="R")
            nc.vector._custom_dve(
                TT_MIN_RED, out=R[:], in0=s[:, 0:2 * QW],
                in1=s[:, 2 * QW:4 * QW], s0=60000.0,
                accum_out=rowmaxes[:, ib:ib + 1],
            )

        psum_ctx.__exit__(None, None, None)

        # ---- tails ----
        tailp = ctx.enter_context(
            tc.tile_pool(name="tailp", bufs=2, space=bass.MemorySpace.PSUM)
        )
        # loss2 partial: sum_i max_j  -> [128,1]
        acc2 = persist.tile([128, 1], F32)
        nc.vector.reduce_sum(out=acc2[:], in_=rowmaxes[:], axis=AX.X)

        # loss1: partition-max of every C column via PE transpose (4 chunks
        # batched per PSUM tile, one [128, 4, 128] reduce each), then sum_j
        n_cols = n // 128
        colmax_cols = persist.tile([128, n_cols], F32)
        for g in range(n_cols // 4):
            pt = tailp.tile([128, 512], F16, tag="pt")
            for c in range(4):
                ch = g * 4 + c
                nc.tensor.transpose(
                    pt[:, c * 128:(c + 1) * 128],
                    C[:, ch * 128:(ch + 1) * 128], ident[:],
                )
            nc.vector.tensor_reduce(
                out=colmax_cols[:, g * 4:g * 4 + 4],
                in_=pt[:].rearrange("p (c f) -> p c f", c=4),
                axis=AX.X, op=ALU.min,
            )
        acc1 = persist.tile([128, 1], F32)
        nc.vector.reduce_sum(out=acc1[:], in_=colmax_cols[:], axis=AX.X)

        total = persist.tile([128, 1], F32)
        nc.vector.tensor_tensor(out=total[:], in0=acc1[:], in1=acc2[:], op=ALU.add)

        # partition-sum via matmul with ones, then scale by -2
        ones = persist.tile([128, 1], F32)
        nc.vector.memset(ones[:], 1.0)
        ps = tailp.tile([1, 1], F32, tag="ps")
        nc.tensor.matmul(ps[:], ones[:], total[:], start=True, stop=True)
        out_sb = persist.tile([1, 1], F32)
        nc.scalar.mul(out_sb[:], ps[:], 2.0)
        nc.sync.dma_start(out_d[:], out_sb[:])

    nc.compile()
    return nc


def kernel(preds: np.ndarray, gts: np.ndarray) -> np.ndarray:
    global _last_results
    assert preds.shape == (B, D, N) and gts.shape == (B, D, N)
    nc = build_kernel(N)
    eye = np.eye(128, dtype=np.float16)
    in_maps = [
        {
            "preds": np.ascontiguousarray(preds[b], dtype=np.float32),
            "gts": np.ascontiguousarray(gts[b], dtype=np.float32),
            "ident": eye,
            "identf": np.eye(32, dtype=np.float32),
        }
        for b in range(N_CORES)
    ]
    res = run_bass_kernel_spmd(
        nc,
        in_maps,
        core_ids=list(range(N_CORES)),
        trace=bool(os.environ.get("BASS_TRACE")),
    )
    _last_results = res
    total = sum(float(res.results[i]["out"].reshape(-1)[0]) for i in range(N_CORES))
    return np.array(total, dtype=np.float32)


# revision 22
# speedup vs baseline: 1.1333x; 1.0042x over previous
"""Chamfer loss kernel for Trainium2 (8 NeuronCores, data-parallel over batch).

Math:
  For each batch b: P[i,j] = |x_i|^2 + |y_j|^2 - 2 x_i.y_j  (x=preds[b].T, y=gts[b].T)
  loss_b = sum_j min_i P + sum_i min_j P ; output = sum_b loss_b.

  On device we compute Z = x.y - |y|^2/2 via a K=11 matmul:
    lhsT rows: [hx0..hx2, hx0..hx2, lx0..lx2, 1, 1]
    rhs  rows: [hy0..hy2, ly0..ly2, hy0..hy2, -hsqy/2, -lsqy/2]
  (bf16 hi/lo pairs give exact cross products; the x.y error from the
  dropped lo.lo term is ~2^-18 relative.)
  The per-row term -|x|^2/2 is applied as a per-partition bias during the
  PSUM exit (ACT activation bias / Pool tensor_scalar), yielding
    s = x.y - |x|^2/2 - |y|^2/2 = -P/2  in fp16.
  min_i P = -2 max_i s, so loss_b = -2 * (sum_j max_i s + sum_i max_j s).

Engine balance (per i-block: 4 PSUM quads = [128, 8192] of Z):
  ACT : fp32->fp16 biased exits of ~7.5 of every 8 quads (2-block cycle)
  Pool: the remaining half-quad exit + col-merge of j in [4096, 8192)
  DVE : col-merge of j in [0, 4096) (fp16 2x) + row pass = one [128,4096]
        TT-max (2x) + one TensorTensorReduce (merge + row-reduce + accum)
  PE  : 4 wide 2048-col matmuls per block; PSUM quads free right after the
        exits so the PE stays busy and ramps to the 2.4 GHz p-state.
"""

import os
from contextlib import ExitStack

import numpy as np

import concourse.bacc as bacc
import concourse.bass as bass
import concourse.dve_ops as dve_ops
import concourse.mybir as mybir
import concourse.tile as tile
from concourse.bass_utils import dve_ver_for, run_bass_kernel_spmd
from concourse.dve_spec import AluOp, C0, Spec, Src0, Src1, _has_src1, lower, minn
from concourse.dve_uop import DveOpSpec


def _register_tt_min_red():
    """Custom DVE op: out = min(in0, in1); accum_out = min(s0, min(out)).

    One instruction covers the whole row pass (pairwise min of two
    half-row streams fused with the free-axis min-reduce). Registered
    into dve_ops at import; the uop table ships inside the NEFF."""
    name = "TT_MIN_RED_ANT"
    if name in dve_ops._SUB_OPCODE_FOR_NAME:
        return next(op for op in dve_ops.OPS if op.name == name)

    def _ref(in0, in1, c0, c1, c2):
        b = np.minimum(in0.astype(np.float32), in1).astype(np.float32)
        acc = np.minimum(c0, b.reshape(b.shape[0], -1).min(axis=-1, keepdims=True))
        return b, acc

    spec = Spec(body=minn(Src0, Src1), accum=AluOp.MIN, accum_init=C0,
                reference=_ref)
    row = max(dve_ops._SUB_OPCODE_FOR_NAME.values()) + 1
    assert row < 0x20
    shas = {}
    for ver in ("v3", "v4"):
        s = DveOpSpec(name=name, opcode=row, uops=lower(spec, ver=ver),
                      rd1_en=_has_src1(spec))
        shas[ver] = s.sha(ver)
    op = dve_ops.DveOp(name, spec, subdim=False, uops_sha=shas)
    dve_ops.OPS.append(op)
    dve_ops.CUSTOM_DVE_SPECS[name] = spec
    dve_ops._SUB_OPCODE_FOR_NAME[name] = row
    return op


TT_MIN_RED = _register_tt_min_red()

B, D, N = 8, 3, 8192
N_CORES = 8

IB = 128          # i-block (output partition dim)
QW = 2048         # PSUM quad width (4 banks, fp32)
N_IB = N // IB    # 64
N_Q = N // QW     # 4

F32 = mybir.dt.float32
F16 = mybir.dt.float16
BF16 = mybir.dt.bfloat16
AX = mybir.AxisListType
ALU = mybir.AluOpType
AF = mybir.ActivationFunctionType

_last_results = None  # stash for test harness (exec_time etc.)


def build_kernel(n: int = N):
    """Builds the SPMD Bass program for one core handling one batch."""
    n_ib = n // IB
    n_q = n // QW

    nc = bacc.Bacc("TRN2", target_bir_lowering=False, debug=False)

    preds_d = nc.dram_tensor("preds", [D, n], F32, kind="ExternalInput").ap()
    gts_d = nc.dram_tensor("gts", [D, n], F32, kind="ExternalInput").ap()
    ident_d = nc.dram_tensor("ident", [128, 128], F16, kind="ExternalInput").ap()
    identf_d = nc.dram_tensor("identf", [32, 32], F32, kind="ExternalInput").ap()
    out_d = nc.dram_tensor("out", [1, 1], F32, kind="ExternalOutput").ap()

    with tile.TileContext(nc) as tc, ExitStack() as ctx:
        persist = ctx.enter_context(tc.tile_pool(name="persist", bufs=1))
        spool = ctx.enter_context(tc.tile_pool(name="spool", bufs=4))
        rpool = ctx.enter_context(tc.tile_pool(name="rpool", bufs=2))

        # ---- persistent tensors ----
        XT = persist.tile([11, n], BF16)
        YT = persist.tile([11, n], BF16)
        ident = persist.tile([128, 128], F16)
        rxh = persist.tile([128, n_ib], F32)     # -|x_i|^2/2, i-major [128, 64]
        C = persist.tile([128, n], F16)          # col-max accumulator over i
        rowmaxes = persist.tile([128, n_ib], F32)
        nc.sync.dma_start(ident[:], ident_d[:])
        # col accumulator starts at +big (gpsimd: off DVE's critical path)
        nc.gpsimd.memset(C[:], float(np.finfo(np.float16).max))

        # ---- prologue ----
        # Math runs in a [96, n/32] layout (partition p = d*32 + c, chunk c
        # of 32) so all lanes are used; DMAs scatter rows into place after.
        fw = n // 32
        with tc.tile_pool(name="propool", bufs=1) as pro:
            # x side: hi/lo split only (no squares; rx handled via bias)
            Px = pro.tile([96, fw], F32)
            Hx = pro.tile([96, fw], BF16)
            Lx = pro.tile([96, fw], BF16)
            nc.sync.dma_start(Px[:], preds_d.rearrange("d (c f) -> (d c) f", c=32))
            nc.scalar.copy(Hx[:], Px[:])
            nc.vector.tensor_tensor(out=Lx[:], in0=Px[:], in1=Hx[:], op=ALU.subtract)

            # y side: hi/lo split + summed squares scaled by -1/2
            Py = pro.tile([96, fw], F32)
            Hy = pro.tile([96, fw], BF16)
            Ly = pro.tile([96, fw], BF16)
            nc.sync.dma_start(Py[:], gts_d.rearrange("d (c f) -> (d c) f", c=32))
            nc.scalar.copy(Hy[:], Py[:])
            nc.vector.tensor_tensor(out=Ly[:], in0=Py[:], in1=Hy[:], op=ALU.subtract)
            # engines need all operands on the same base partition, so the
            # per-d sum runs in a [32, (d, fw)] layout loaded separately
            Yd = pro.tile([32, 3 * fw], F32)
            for d in range(D):
                nc.sync.dma_start(
                    Yd[:, d * fw:(d + 1) * fw],
                    gts_d[d:d + 1, :].rearrange("o (c f) -> (o c) f", c=32),
                )
            SQ = pro.tile([32, 3 * fw], F32)
            SY = pro.tile([32, fw], F32)
            S2 = pro.tile([32, fw], F32)
            HS = pro.tile([32, fw], BF16)
            LS = pro.tile([32, fw], BF16)
            nc.vector.tensor_tensor(out=SQ[:], in0=Yd[:], in1=Yd[:], op=ALU.mult)
            nc.vector.tensor_reduce(
                out=SY[:], in_=SQ[:].rearrange("p (d f) -> p f d", d=3),
                axis=AX.X, op=ALU.add,
            )
            nc.scalar.mul(S2[:], SY[:], -0.5)
            nc.scalar.copy(HS[:], S2[:])
            nc.vector.tensor_tensor(out=LS[:], in0=S2[:], in1=HS[:], op=ALU.subtract)

            # ones rows for the -|y|^2/2 rank-1 terms
            ONE = pro.tile([64, fw], BF16)
            nc.gpsimd.memset(ONE[:], 1.0)

            # scatter into XT/YT row layout (batched 3-row DMAs)
            def scat(T, r0, nrows, src):
                nc.sync.dma_start(
                    T[r0:r0 + nrows, :].rearrange("p (c f) -> p c f", c=32),
                    src,
                )
            scat(XT, 0, 3, Hx[:])
            scat(XT, 3, 3, Hx[:])
            scat(XT, 6, 3, Lx[:])
            scat(XT, 9, 2, ONE[:])
            scat(YT, 0, 3, Hy[:])
            scat(YT, 3, 3, Ly[:])
            scat(YT, 6, 3, Hy[:])
            scat(YT, 9, 1, HS[:])
            scat(YT, 10, 1, LS[:])

            # rxh[p, b] = -|x_{b*128+p}|^2/2  (i-major for per-partition bias).
            # Built via PE transpose: |x|^2 in the chunk layout [32, 256]
            # (i = c*256 + f), transposed 128-col halves give [128, 32]
            # tiles whose cols are even/odd block indices.
            Xd = pro.tile([32, 3 * fw], F32)
            for d in range(D):
                nc.sync.dma_start(
                    Xd[:, d * fw:(d + 1) * fw],
                    preds_d[d:d + 1, :].rearrange("o (c f) -> (o c) f", c=32),
                )
            XSQ = pro.tile([32, 3 * fw], F32)
            RXS = pro.tile([32, fw], F32)
            nc.vector.tensor_tensor(out=XSQ[:], in0=Xd[:], in1=Xd[:], op=ALU.mult)
            nc.vector.tensor_reduce(
                out=RXS[:], in_=XSQ[:].rearrange("p (d f) -> p f d", d=3),
                axis=AX.X, op=ALU.add,
            )
            identf = pro.tile([32, 32], F32)
            nc.sync.dma_start(identf[:], identf_d[:])
            with tc.tile_pool(name="prot", bufs=1, space=bass.MemorySpace.PSUM) as prot:
                for h in range(2):
                    pt = prot.tile([128, 32], F32, name=f"pt{h}")
                    nc.tensor.transpose(
                        pt[:], RXS[:, h * 128:(h + 1) * 128], identf[:])
                    nc.scalar.mul(
                        rxh[:].rearrange("p (b two) -> p two b", two=2)[:, h, :],
                        pt[:], 0.5)

        # ---- main loop ----
        psum_ctx = tc.tile_pool(name="psum", bufs=2, space=bass.MemorySpace.PSUM)
        psum = psum_ctx.__enter__()
        NEG = float(60000.0)
        for ib in range(n_ib):
            lhsT = XT[:, ib * IB:(ib + 1) * IB]
            bias = rxh[:, ib:ib + 1]
            s = spool.tile([128, n], F16, tag="s")
            for q in range(n_q):
                p = psum.tile([128, QW], F32, tag="p")
                for m in range(QW // 512):
                    c0 = q * QW + m * 512
                    nc.tensor.matmul(
                        p[:, m * 512:(m + 1) * 512], lhsT, YT[:, c0:c0 + 512],
                        start=True, stop=True,
                    )
                # ACT exits the quad (Pool can't read PSUM on TRN2)
                nc.scalar.activation(
                    s[:, q * QW:(q + 1) * QW], p[:],
                    AF.Identity, bias=bias, scale=-1.0,
                )
            # col accumulator: one wide fp16 2x merge (DVE is the only
            # engine that can do bulk max on TRN2; Pool TT doesn't lower)
            nc.vector.tensor_tensor(
                out=C[:], in0=C[:], in1=s[:], op=ALU.min)
            # row pass: one custom DVE op (pairwise min + min-reduce accum)
            R = rpool.tile([128, 2 * QW], F16, tag="R")
            nc.vector._custom_dve(
                TT_MIN_RED, out=R[:], in0=s[:, 0:2 * QW],
                in1=s[:, 2 * QW:4 * QW], s0=60000.0,
                accum_out=rowmaxes[:, ib:ib + 1],
            )

        psum_ctx.__exit__(None, None, None)

        # ---- tails ----
        tailp = ctx.enter_context(
            tc.tile_pool(name="tailp", bufs=2, space=bass.MemorySpace.PSUM)
        )
        # loss2 partial: sum_i max_j  -> [128,1]
        acc2 = persist.tile([128, 1], F32)
        nc.vector.reduce_sum(out=acc2[:], in_=rowmaxes[:], axis=AX.X)

        # loss1: partition-max of every C column via PE transpose (4 chunks
        # batched per PSUM tile, one [128, 4, 128] reduce each), then sum_j
        n_cols = n // 128
        colmax_cols = persist.tile([128, n_cols], F32)
        for g in range(n_cols // 4):
            pt = tailp.tile([128, 512], F16, tag="pt")
            for c in range(4):
                ch = g * 4 + c
                nc.tensor.transpose(
                    pt[:, c * 128:(c + 1) * 128],
                    C[:, ch * 128:(ch + 1) * 128], ident[:],
                )
            nc.vector.tensor_reduce(
                out=colmax_cols[:, g * 4:g * 4 + 4],
                in_=pt[:].rearrange("p (c f) -> p c f", c=4),
                axis=AX.X, op=ALU.min,
            )
        acc1 = persist.tile([128, 1], F32)
        nc.vector.reduce_sum(out=acc1[:], in_=colmax_cols[:], axis=AX.X)

        total = persist.tile([128, 1], F32)
        nc.vector.tensor_tensor(out=total[:], in0=acc1[:], in1=acc2[:], op=ALU.add)

        # partition-sum via matmul with ones, then scale by -2
        ones = persist.tile([128, 1], F32)
        nc.vector.memset(ones[:], 1.0)
        ps = tailp.tile([1, 1], F32, tag="ps")
        nc.tensor.matmul(ps[:], ones[:], total[:], start=True, stop=True)
        out_sb = persist.tile([1, 1], F32)
        nc.scalar.mul(out_sb[:], ps[:], 2.0)
        nc.sync.dma_start(out_d[:], out_sb[:])

    nc.compile()
    return nc


def kernel(preds: np.ndarray, gts: np.ndarray) -> np.ndarray:
    global _last_results
    assert preds.shape == (B, D, N) and gts.shape == (B, D, N)
    nc = build_kernel(N)
    eye = np.eye(128, dtype=np.float16)
    in_maps = [
        {
            "preds": np.ascontiguousarray(preds[b], dtype=np.float32),
            "gts": np.ascontiguousarray(gts[b], dtype=np.float32),
            "ident": eye,
            "identf": np.eye(32, dtype=np.float32),
        }
        for b in range(N_CORES)
    ]
    res = run_bass_kernel_spmd(
        nc,
        in_maps,
        core_ids=list(range(N_CORES)),
        trace=bool(os.environ.get("BASS_TRACE")),
    )
    _last_results = res
    total = sum(float(res.results[i]["out"].reshape(-1)[0]) for i in range(N_CORES))
    return np.array(total, dtype=np.float32)


# revision 23
# speedup vs baseline: 1.9011x; 1.6776x over previous
"""Chamfer loss kernel for Trainium2 (8 NeuronCores, data-parallel over batch).

Math:
  For each batch b: P[i,j] = |x_i|^2 + |y_j|^2 - 2 x_i.y_j  (x=preds[b].T, y=gts[b].T)
  loss_b = sum_j min_i P + sum_i min_j P ; output = sum_b loss_b.

  On device we compute Z = x.y - |y|^2/2 via a K=11 matmul:
    lhsT rows: [hx0..hx2, hx0..hx2, lx0..lx2, 1, 1]
    rhs  rows: [hy0..hy2, ly0..ly2, hy0..hy2, -hsqy/2, -lsqy/2]
  (bf16 hi/lo pairs give exact cross products; the x.y error from the
  dropped lo.lo term is ~2^-18 relative.)
  The per-row term -|x|^2/2 is applied as a per-partition bias during the
  PSUM exit (ACT activation bias / Pool tensor_scalar), yielding
    s = x.y - |x|^2/2 - |y|^2/2 = -P/2  in fp16.
  min_i P = -2 max_i s, so loss_b = -2 * (sum_j max_i s + sum_i max_j s).

Engine balance (per i-block: 4 PSUM quads = [128, 8192] of Z):
  ACT : fp32->fp16 biased exits of ~7.5 of every 8 quads (2-block cycle)
  Pool: the remaining half-quad exit + col-merge of j in [4096, 8192)
  DVE : col-merge of j in [0, 4096) (fp16 2x) + row pass = one [128,4096]
        TT-max (2x) + one TensorTensorReduce (merge + row-reduce + accum)
  PE  : 4 wide 2048-col matmuls per block; PSUM quads free right after the
        exits so the PE stays busy and ramps to the 2.4 GHz p-state.
"""

import os
from contextlib import ExitStack

import numpy as np

import concourse.bacc as bacc
import concourse.bass as bass
import concourse.dve_ops as dve_ops
import concourse.mybir as mybir
import concourse.tile as tile
from concourse.bass_utils import dve_ver_for, run_bass_kernel_spmd
from concourse.dve_spec import AluOp, C0, Spec, Src0, Src1, _has_src1, lower, minn
from concourse.dve_uop import DveOpSpec


def _register_tt_min_red():
    """Custom DVE op: out = min(in0, in1); accum_out = min(s0, min(out)).

    One instruction covers the whole row pass (pairwise min of two
    half-row streams fused with the free-axis min-reduce). Registered
    into dve_ops at import; the uop table ships inside the NEFF."""
    name = "TT_MIN_RED_ANT"
    if name in dve_ops._SUB_OPCODE_FOR_NAME:
        return next(op for op in dve_ops.OPS if op.name == name)

    def _ref(in0, in1, c0, c1, c2):
        b = np.minimum(in0.astype(np.float32), in1).astype(np.float32)
        acc = np.minimum(c0, b.reshape(b.shape[0], -1).min(axis=-1, keepdims=True))
        return b, acc

    spec = Spec(body=minn(Src0, Src1), accum=AluOp.MIN, accum_init=C0,
                reference=_ref)
    row = max(dve_ops._SUB_OPCODE_FOR_NAME.values()) + 1
    assert row < 0x20
    shas = {}
    for ver in ("v3", "v4"):
        s = DveOpSpec(name=name, opcode=row, uops=lower(spec, ver=ver),
                      rd1_en=_has_src1(spec))
        shas[ver] = s.sha(ver)
    op = dve_ops.DveOp(name, spec, subdim=False, uops_sha=shas)
    dve_ops.OPS.append(op)
    dve_ops.CUSTOM_DVE_SPECS[name] = spec
    dve_ops._SUB_OPCODE_FOR_NAME[name] = row
    return op


TT_MIN_RED = _register_tt_min_red()

B, D, N = 8, 3, 8192
N_CORES = 8

IB = 128          # i-block (output partition dim)
QW = 2048         # PSUM quad width (4 banks, fp32)
N_IB = N // IB    # 64
N_Q = N // QW     # 4

F32 = mybir.dt.float32
F16 = mybir.dt.float16
BF16 = mybir.dt.bfloat16
AX = mybir.AxisListType
ALU = mybir.AluOpType
AF = mybir.ActivationFunctionType

_last_results = None  # stash for test harness (exec_time etc.)


def build_kernel(n: int = N):
    """Builds the SPMD Bass program for one core handling one batch."""
    n_ib = n // IB
    n_q = n // QW

    nc = bacc.Bacc("TRN2", target_bir_lowering=False, debug=False)

    preds_d = nc.dram_tensor("preds", [D, n], F32, kind="ExternalInput").ap()
    gts_d = nc.dram_tensor("gts", [D, n], F32, kind="ExternalInput").ap()
    ident_d = nc.dram_tensor("ident", [128, 128], F16, kind="ExternalInput").ap()
    identf_d = nc.dram_tensor("identf", [32, 32], F32, kind="ExternalInput").ap()
    out_d = nc.dram_tensor("out", [1, 1], F32, kind="ExternalOutput").ap()

    with tile.TileContext(nc) as tc, ExitStack() as ctx:
        persist = ctx.enter_context(tc.tile_pool(name="persist", bufs=1))
        spool = ctx.enter_context(tc.tile_pool(name="spool", bufs=4))
        rpool = ctx.enter_context(tc.tile_pool(name="rpool", bufs=2))

        # ---- persistent tensors ----
        XT = persist.tile([11, n], BF16)
        YT = persist.tile([11, n], BF16)
        ident = persist.tile([128, 128], F16)
        rxh = persist.tile([128, n_ib], F32)     # -|x_i|^2/2, i-major [128, 64]
        C = persist.tile([128, n], F16)          # col-max accumulator over i
        rowmaxes = persist.tile([128, n_ib], F32)
        nc.sync.dma_start(ident[:], ident_d[:])
        # col accumulator starts at +big (gpsimd: off DVE's critical path)
        nc.gpsimd.memset(C[:], float(np.finfo(np.float16).max))

        # ---- prologue ----
        # Math runs in a [96, n/32] layout (partition p = d*32 + c, chunk c
        # of 32) so all lanes are used; DMAs scatter rows into place after.
        fw = n // 32
        with tc.tile_pool(name="propool", bufs=1) as pro:
            # x side: hi/lo split only (no squares; rx handled via bias)
            Px = pro.tile([96, fw], F32)
            Hx = pro.tile([96, fw], BF16)
            Lx = pro.tile([96, fw], BF16)
            nc.sync.dma_start(Px[:], preds_d.rearrange("d (c f) -> (d c) f", c=32))
            nc.scalar.copy(Hx[:], Px[:])
            nc.vector.tensor_tensor(out=Lx[:], in0=Px[:], in1=Hx[:], op=ALU.subtract)

            # y side: hi/lo split + summed squares scaled by -1/2
            Py = pro.tile([96, fw], F32)
            Hy = pro.tile([96, fw], BF16)
            Ly = pro.tile([96, fw], BF16)
            nc.scalar.dma_start(Py[:], gts_d.rearrange("d (c f) -> (d c) f", c=32))
            nc.scalar.copy(Hy[:], Py[:])
            nc.vector.tensor_tensor(out=Ly[:], in0=Py[:], in1=Hy[:], op=ALU.subtract)
            # engines need all operands on the same base partition, so the
            # per-d sum runs in a [32, (d, fw)] layout loaded separately
            Yd = pro.tile([32, 3 * fw], F32)
            for d in range(D):
                nc.scalar.dma_start(
                    Yd[:, d * fw:(d + 1) * fw],
                    gts_d[d:d + 1, :].rearrange("o (c f) -> (o c) f", c=32),
                )
            SQ = pro.tile([32, 3 * fw], F32)
            SY = pro.tile([32, fw], F32)
            S2 = pro.tile([32, fw], F32)
            HS = pro.tile([32, fw], BF16)
            LS = pro.tile([32, fw], BF16)
            nc.vector.tensor_tensor(out=SQ[:], in0=Yd[:], in1=Yd[:], op=ALU.mult)
            nc.vector.tensor_reduce(
                out=SY[:], in_=SQ[:].rearrange("p (d f) -> p f d", d=3),
                axis=AX.X, op=ALU.add,
            )
            nc.scalar.mul(S2[:], SY[:], -0.5)
            nc.scalar.copy(HS[:], S2[:])
            nc.vector.tensor_tensor(out=LS[:], in0=S2[:], in1=HS[:], op=ALU.subtract)

            # ones rows for the -|y|^2/2 rank-1 terms
            ONE = pro.tile([64, fw], BF16)
            nc.gpsimd.memset(ONE[:], 1.0)

            # scatter into XT/YT row layout (batched 3-row DMAs)
            def scat(eng, T, r0, nrows, src):
                eng.dma_start(
                    T[r0:r0 + nrows, :].rearrange("p (c f) -> p c f", c=32),
                    src,
                )
            scat(nc.sync, XT, 0, 3, Hx[:])
            scat(nc.sync, XT, 3, 3, Hx[:])
            scat(nc.sync, XT, 6, 3, Lx[:])
            scat(nc.sync, XT, 9, 2, ONE[:])
            scat(nc.scalar, YT, 0, 3, Hy[:])
            scat(nc.scalar, YT, 3, 3, Ly[:])
            scat(nc.scalar, YT, 6, 3, Hy[:])
            scat(nc.scalar, YT, 9, 1, HS[:])
            scat(nc.scalar, YT, 10, 1, LS[:])

            # rxh[p, b] = -|x_{b*128+p}|^2/2  (i-major for per-partition bias).
            # Built via PE transpose: |x|^2 in the chunk layout [32, 256]
            # (i = c*256 + f), transposed 128-col halves give [128, 32]
            # tiles whose cols are even/odd block indices.
            Xd = pro.tile([32, 3 * fw], F32)
            for d in range(D):
                nc.sync.dma_start(
                    Xd[:, d * fw:(d + 1) * fw],
                    preds_d[d:d + 1, :].rearrange("o (c f) -> (o c) f", c=32),
                )
            XSQ = pro.tile([32, 3 * fw], F32)
            RXS = pro.tile([32, fw], F32)
            nc.vector.tensor_tensor(out=XSQ[:], in0=Xd[:], in1=Xd[:], op=ALU.mult)
            nc.vector.tensor_reduce(
                out=RXS[:], in_=XSQ[:].rearrange("p (d f) -> p f d", d=3),
                axis=AX.X, op=ALU.add,
            )
            identf = pro.tile([32, 32], F32)
            nc.sync.dma_start(identf[:], identf_d[:])
            with tc.tile_pool(name="prot", bufs=1, space=bass.MemorySpace.PSUM) as prot:
                for h in range(2):
                    pt = prot.tile([128, 32], F32, name=f"pt{h}")
                    nc.tensor.transpose(
                        pt[:], RXS[:, h * 128:(h + 1) * 128], identf[:])
                    nc.scalar.mul(
                        rxh[:].rearrange("p (b two) -> p two b", two=2)[:, h, :],
                        pt[:], 0.5)

        # ---- main loop ----
        psum_ctx = tc.tile_pool(name="psum", bufs=2, space=bass.MemorySpace.PSUM)
        psum = psum_ctx.__enter__()
        NEG = float(60000.0)
        for ib in range(n_ib):
            lhsT = XT[:, ib * IB:(ib + 1) * IB]
            bias = rxh[:, ib:ib + 1]
            s = spool.tile([128, n], F16, tag="s")
            for q in range(n_q):
                p = psum.tile([128, QW], F32, tag="p")
                for m in range(QW // 512):
                    c0 = q * QW + m * 512
                    nc.tensor.matmul(
                        p[:, m * 512:(m + 1) * 512], lhsT, YT[:, c0:c0 + 512],
                        start=True, stop=True,
                    )
                # ACT exits the quad (Pool can't read PSUM on TRN2)
                nc.scalar.activation(
                    s[:, q * QW:(q + 1) * QW], p[:],
                    AF.Identity, bias=bias, scale=-1.0,
                )
            # col accumulator: one wide fp16 2x merge (DVE is the only
            # engine that can do bulk max on TRN2; Pool TT doesn't lower)
            nc.vector.tensor_tensor(
                out=C[:], in0=C[:], in1=s[:], op=ALU.min)
            # row pass: one custom DVE op (pairwise min + min-reduce accum)
            R = rpool.tile([128, 2 * QW], F16, tag="R")
            nc.vector._custom_dve(
                TT_MIN_RED, out=R[:], in0=s[:, 0:2 * QW],
                in1=s[:, 2 * QW:4 * QW], s0=60000.0,
                accum_out=rowmaxes[:, ib:ib + 1],
            )

        psum_ctx.__exit__(None, None, None)

        # ---- tails ----
        tailp = ctx.enter_context(
            tc.tile_pool(name="tailp", bufs=2, space=bass.MemorySpace.PSUM)
        )
        # loss2 partial: sum_i max_j  -> [128,1]
        acc2 = persist.tile([128, 1], F32)
        nc.vector.reduce_sum(out=acc2[:], in_=rowmaxes[:], axis=AX.X)

        # loss1: partition-max of every C column via PE transpose (4 chunks
        # batched per PSUM tile, one [128, 4, 128] reduce each), then sum_j
        n_cols = n // 128
        colmax_cols = persist.tile([128, n_cols], F32)
        for g in range(n_cols // 4):
            pt = tailp.tile([128, 512], F16, tag="pt")
            for c in range(4):
                ch = g * 4 + c
                nc.tensor.transpose(
                    pt[:, c * 128:(c + 1) * 128],
                    C[:, ch * 128:(ch + 1) * 128], ident[:],
                )
            nc.vector.tensor_reduce(
                out=colmax_cols[:, g * 4:g * 4 + 4],
                in_=pt[:].rearrange("p (c f) -> p c f", c=4),
                axis=AX.X, op=ALU.min,
            )
        acc1 = persist.tile([128, 1], F32)
        nc.vector.reduce_sum(out=acc1[:], in_=colmax_cols[:], axis=AX.X)

        total = persist.tile([128, 1], F32)
        nc.vector.tensor_tensor(out=total[:], in0=acc1[:], in1=acc2[:], op=ALU.add)

        # partition-sum via matmul with ones, then scale by -2
        ones = persist.tile([128, 1], F32)
        nc.vector.memset(ones[:], 1.0)
        ps = tailp.tile([1, 1], F32, tag="ps")
        nc.tensor.matmul(ps[:], ones[:], total[:], start=True, stop=True)
        out_sb = persist.tile([1, 1], F32)
        nc.scalar.mul(out_sb[:], ps[:], 2.0)
        nc.sync.dma_start(out_d[:], out_sb[:])

    nc.compile()
    return nc


def kernel(preds: np.ndarray, gts: np.ndarray) -> np.ndarray:
    global _last_results
    assert preds.shape == (B, D, N) and gts.shape == (B, D, N)
    nc = build_kernel(N)
    eye = np.eye(128, dtype=np.float16)
    in_maps = [
        {
            "preds": np.ascontiguousarray(preds[b], dtype=np.float32),
            "gts": np.ascontiguousarray(gts[b], dtype=np.float32),
            "ident": eye,
            "identf": np.eye(32, dtype=np.float32),
        }
        for b in range(N_CORES)
    ]
    res = run_bass_kernel_spmd(
        nc,
        in_maps,
        core_ids=list(range(N_CORES)),
        trace=bool(os.environ.get("BASS_TRACE")),
    )
    _last_results = res
    total = sum(float(res.results[i]["out"].reshape(-1)[0]) for i in range(N_CORES))
    return np.array(total, dtype=np.float32)
